# revision 6
# baseline (speedup 1.0000x reference)
"""Trainium2 Bass kernel for a transformer decoder layer (self-attn +
cross-attn + FFN) on 8 NeuronCores, zero collectives.

Sharding: data-parallel. Core c (0..7) owns batch b = c//4 and four
query subtiles qi in {j, 4+j, 8+j, 12+j} (j = c%4, 128 rows each) of
that batch — the stride-4 interleave balances the causal-attention
work across cores. Every core recomputes the full K/V projections for
its batch (2048 rows), so no inter-core communication is needed. The
compiled program is identical on all cores (SPMD); causal masking and
the per-core query positions live entirely in the data.

Performance layout (v2):
- All projection/AV/FFN matmuls run fp8e4 inputs with DoubleRow perf
  mode (two 128-row k-tiles contracted per instruction at 0.5 cyc/col)
  and fp32 PSUM accumulation. Scores stay bf16 (dk=64 contraction has
  no DoubleRow form).
- Weights are pre-scaled x64 on the host so sigma~0.02 values sit in
  fp8e4's normal range; activations carry a x4096 scale through the
  residual stream (LayerNorm is scale-invariant, so the rescale folds
  into the rstd computation for free). V's x64 cancels against the
  fp8 dynamic-range scale of the attention output, Q's folds into the
  existing 1/sqrt(dk) multiply.
- Scores are computed transposed (S^T [k, q]); exp runs on the
  activation engine writing fp8 directly; softmax skips the max trick
  (|scores| < ~2; masked entries get additive -30 via PE bias matmul).
- The softmax denominator l[q] rides as a 65th "ones" column of V; its
  reciprocal is taken in place at PSUM partition 64 and broadcast with
  a K=1 matmul from that partition — no cross-partition DMA.
- Cross-attention K/V projection is emitted interleaved with the
  self-attention head loop so the PE stays busy while attention is
  activation-engine-bound.
"""

import contextlib

import numpy as np
import ml_dtypes

import concourse.bass as bass
import concourse.tile as tile
from concourse import mybir
from concourse.bass import ds
from concourse.bass_utils import run_bass_kernel_spmd

B, S, S_ENC, D, H, DK, DFF = 2, 2048, 2048, 512, 8, 64, 2048
EPS = 1e-5
NCORES = 8
QSUB = 4          # query subtiles per core (128 rows each)
KT_CA = 16        # cross-attention key tiles (128 keys each)
MASK_NEG = -30.0  # additive mask value (see module docstring)
WS = 64.0         # fp8 weight scale
XS = WS * WS      # activation carry scale (4096)

F32 = mybir.dt.float32
BF16 = mybir.dt.bfloat16
FP8 = mybir.dt.float8e4
DR = mybir.MatmulPerfMode.DoubleRow


# ---------------------------------------------------------------------------
# walrus legalization: this neuronxcc build rejects instructions carrying
# more than one sync wait. Tile attaches several to the kernel-tail Drain
# (and occasionally elsewhere); hoist the extras onto same-engine NOPs.
# ---------------------------------------------------------------------------
def _split_multiwaits(nc):
    nopid = 0
    for fn in nc.m.functions:
        for blk in fn.blocks:
            insts = blk.instructions
            i = 0
            while i < len(insts):
                inst = insts[i]
                si = getattr(inst, "sync_info", None)
                if si is not None and len(si.on_wait) > 1:
                    waits = list(si.on_wait)
                    inst.sync_info = mybir.SyncInfo(
                        on_wait=[waits[-1]], on_update=list(si.on_update)
                    )
                    for w in waits[:-1]:
                        nop = mybir.InstNoOp(
                            name=f"I-waitsplit-{nopid}",
                            engine=inst.engine,
                            sync_info=mybir.SyncInfo(on_wait=[w], on_update=[]),
                            bass_nofuse=True,
                        )
                        nopid += 1
                        insts.insert(i, nop)
                        i += 1
                i += 1


class _TileContext(tile.TileContext):
    def __exit__(self, exc_type, exc, tb):
        ret = super().__exit__(exc_type, exc, tb)
        if exc_type is None:
            _split_multiwaits(self.nc)
        return ret


def _bcast_dram(dram_ap, parts=128):
    """AP reading a 1-D DRAM vector replicated across `parts` partitions."""
    return bass.AP(
        tensor=dram_ap.tensor,
        offset=dram_ap.offset,
        ap=[[0, parts]] + list(dram_ap.ap),
    )


# ---------------------------------------------------------------------------
# program builder (identical for every core; all core differences are data)
# ---------------------------------------------------------------------------
def build_program(ln_identity, sa_all_bias, ca_kbias):
    nc = bass.Bass()

    inp = {}

    def dram(name, shape, dt):
        inp[name] = nc.declare_dram_parameter(name, list(shape), dt, isOutput=False)
        return inp[name]

    dram("identity", (128, 128), BF16)
    dram("decT", (D, S), FP8)
    dram("qT0", (D, 512), FP8)
    dram("resid0", (512, D), F32)         # XS*(dec rows + bv@wo + bo)
    dram("encT", (D, S_ENC), FP8)
    for wnm, shp in [
        ("w_sa_q", (D, D)), ("w_sa_k", (D, D)), ("w_sa_v", (D, D)),
        ("w_ca_q", (D, D)), ("w_ca_k", (D, D)), ("w_ca_v", (D, D)),
        ("w_ff1", (D, DFF)), ("w_ff2", (DFF, D)),
    ]:
        dram(wnm, shp, FP8)
    dram("w_sa_o8", (DK, H, D), FP8)   # 64*wo reshaped to [64, head, 512]
    dram("w_ca_o8", (DK, H, D), FP8)
    dram("bq_sa", (D,), F32)      # 64*bq
    dram("bq_ca", (D,), F32)
    dram("bo_ca", (D,), F32)      # XS*(bv@wo + bo)
    dram("bff1", (DFF,), F32)     # 64*b1
    dram("bff2", (D,), F32)       # XS*b2
    ngb = 4 if sa_all_bias else 1
    # [slot, group, kt-in-group, q 128, k 128] bf16, [q, k] orientation
    dram("sa_bias", (QSUB, ngb, 4, 128, 128), BF16)
    if ca_kbias:
        dram("ca_kb", (KT_CA, 128), F32)
    if not ln_identity:
        for i in (1, 2, 3):
            dram(f"ln{i}_g", (D,), F32)
            dram(f"ln{i}_b", (D,), F32)   # ln1/ln2 host-scaled by XS

    out_y = nc.declare_dram_parameter("y", [512, D], F32, isOutput=True)

    with _TileContext(nc) as tc:
        with contextlib.ExitStack() as ctx:
            const = ctx.enter_context(tc.tile_pool(name="const", bufs=1))
            xt = ctx.enter_context(tc.tile_pool(name="xt", bufs=1))
            wp = ctx.enter_context(tc.tile_pool(name="wp", bufs=1))
            res = ctx.enter_context(tc.tile_pool(name="res", bufs=1))
            work = ctx.enter_context(tc.tile_pool(name="work", bufs=2))
            xbfp = ctx.enter_context(tc.tile_pool(name="xbfp", bufs=1))
            attn = ctx.enter_context(tc.tile_pool(name="attn", bufs=1))
            expp = ctx.enter_context(tc.tile_pool(name="expp", bufs=3))
            rhp = ctx.enter_context(tc.tile_pool(name="rhp", bufs=2))
            ps = ctx.enter_context(tc.tile_pool(name="ps", bufs=2, space="PSUM"))
            ps_s = ctx.enter_context(tc.tile_pool(name="ps_s", bufs=2, space="PSUM"))
            ps_o = ctx.enter_context(tc.tile_pool(name="ps_o", bufs=2, space="PSUM"))

            # ---- constants ----
            ident = const.tile([128, 128], BF16)
            nc.sync.dma_start(out=ident, in_=inp["identity"][:])
            eps_t = const.tile([128, 1], F32)
            nc.vector.memset(eps_t, EPS)
            eps3_t = const.tile([128, 1], F32)
            nc.vector.memset(eps3_t, EPS * XS * XS)
            ones65 = const.tile([DK + 1, DK], BF16)
            nc.vector.memset(ones65, 1.0)

            def load_bc(name):
                t = const.tile([128, D], F32, tag=f"bc_{name}")
                nc.sync.dma_start(out=t, in_=_bcast_dram(inp[name][:]))
                return t

            bo_ca_bc = load_bc("bo_ca")
            bff2_bc = load_bc("bff2")
            ln_bc = {}
            if not ln_identity:
                for i in (1, 2, 3):
                    ln_bc[i] = (load_bc(f"ln{i}_g"), load_bc(f"ln{i}_b"))

            bq_sa_sb = const.tile([128, 4], F32)
            nc.sync.dma_start(
                out=bq_sa_sb, in_=inp["bq_sa"][:].rearrange("(g p) -> p g", p=128)
            )
            bq_ca_sb = const.tile([128, 4], F32)
            nc.sync.dma_start(
                out=bq_ca_sb, in_=inp["bq_ca"][:].rearrange("(g p) -> p g", p=128)
            )
            bff1_sb = const.tile([128, 16], F32)
            nc.sync.dma_start(
                out=bff1_sb, in_=inp["bff1"][:].rearrange("(c p) -> p c", p=128)
            )

            sa_bias_sb = const.tile([128, QSUB, ngb, 4, 128], BF16)
            nc.sync.dma_start(
                out=sa_bias_sb,
                in_=inp["sa_bias"][:].rearrange("s g t p k -> p s g t k"),
            )
            if ca_kbias:
                ca_kb_sb = const.tile([128, KT_CA], F32)
                nc.sync.dma_start(
                    out=ca_kb_sb, in_=inp["ca_kb"][:].rearrange("t p -> p t")
                )

            x1 = res.tile([128, QSUB, D], F32, tag="x1")
            x2 = res.tile([128, QSUB, D], F32, tag="x2")
            resid0_sb = res.tile([128, QSUB, D], F32, tag="r0")
            nc.sync.dma_start(
                out=resid0_sb,
                in_=inp["resid0"][:].rearrange("(s p) d -> p s d", p=128),
            )

            # =============================================================
            def kv_projection_rg(KT_t, V_t, srcT_sb, wk_sb, wv_sb, rg):
                """One 512-key row group of the K/V projections (DoubleRow)."""
                for go in range(4):
                    psum = ps.tile([128, 512], F32, tag="psg")
                    for gp in range(2):
                        nc.tensor.matmul(
                            psum, wk_sb[:, 2 * gp:2 * gp + 2, ds(go * 128, 128)],
                            srcT_sb[:, 2 * gp:2 * gp + 2, ds(rg * 512, 512)],
                            start=(gp == 0), stop=(gp == 1), perf_mode=DR,
                        )
                    nc.vector.tensor_copy(
                        out=KT_t[:, go, ds(rg * 512, 512)], in_=psum
                    )
                for k2 in range(4):
                    kc = rg * 4 + k2
                    psum = ps.tile([128, 512], F32, tag="psg")
                    for gp in range(2):
                        nc.tensor.matmul(
                            psum, srcT_sb[:, 2 * gp:2 * gp + 2, ds(kc * 128, 128)],
                            wv_sb[:, 2 * gp:2 * gp + 2, :],
                            start=(gp == 0), stop=(gp == 1), perf_mode=DR,
                        )
                    nc.vector.tensor_copy(
                        out=V_t[:, kc, :, 0:DK],
                        in_=psum.rearrange("p (h d) -> p h d", h=H),
                    )

            def q_projection(QT_t, q_rhs_sb, wq_sb, bq_sb):
                for go in range(4):
                    psum = ps.tile([128, 512], F32, tag="psg")
                    for gp in range(2):
                        nc.tensor.matmul(
                            psum, wq_sb[:, 2 * gp:2 * gp + 2, ds(go * 128, 128)],
                            q_rhs_sb[:, 2 * gp:2 * gp + 2, :],
                            start=(gp == 0), stop=(gp == 1), perf_mode=DR,
                        )
                    # QT = (64*(wq^T x) + 64*bq) / (64*64*sqrt(dk)) = q/512
                    nc.vector.tensor_scalar(
                        out=QT_t[:, go, :], in0=psum,
                        scalar1=bq_sb[:, go:go + 1],
                        scalar2=1.0 / (WS * WS * np.sqrt(DK)),
                        op0=mybir.AluOpType.add, op1=mybir.AluOpType.mult,
                    )

            def layer_norm(src_sb, dst_ap, ln_idx):
                """src = XS*u; LN1/2 write XS*ln(u), LN3 writes ln(u)."""
                stats = work.tile([128, 6], F32, tag="lnstats")
                nc.vector.bn_stats(out=stats, in_=src_sb)
                mv = work.tile([128, 2], F32, tag="lnmv")
                nc.vector.bn_aggr(out=mv, in_=stats)
                rstd = work.tile([128, 1], F32, tag="lnrstd")
                last = ln_idx == 3
                nc.scalar.activation(
                    out=rstd, in_=mv[:, 1:2],
                    func=mybir.ActivationFunctionType.Sqrt,
                    bias=eps3_t if last else eps_t,
                    scale=1.0 if last else 1.0 / (XS * XS),
                )
                nc.vector.reciprocal(out=rstd, in_=rstd)
                nc.gpsimd.tensor_scalar(
                    out=dst_ap, in0=src_sb,
                    scalar1=mv[:, 0:1], scalar2=rstd,
                    op0=mybir.AluOpType.subtract, op1=mybir.AluOpType.mult,
                )
                if not ln_identity:
                    g_bc, b_bc = ln_bc[ln_idx]
                    nc.gpsimd.tensor_tensor(
                        out=dst_ap, in0=dst_ap, in1=g_bc, op=mybir.AluOpType.mult
                    )
                    nc.gpsimd.tensor_tensor(
                        out=dst_ap, in0=dst_ap, in1=b_bc, op=mybir.AluOpType.add
                    )

            def attention(KT, V, QT, attnT, mask_mode, use_ca_kbias, wo8_sb,
                          bo_bc, resid_sb, x_out, ln_idx, filler):
                """mask_mode: 'causal' | 'allbias' | 'none'.
                filler: list of thunks; one is emitted after each head-pair
                to interleave independent PE work with the ACT-bound loop."""
                n_kt = 16
                for hp in range(H // 2):
                    h2 = (2 * hp, 2 * hp + 1)
                    gh = hp
                    psum_os = [
                        ps_o.tile([DK + 1, 512], F32, tag="po", name=f"po_{hp}_{i}")
                        for i in range(2)
                    ]
                    for g in range(4):
                        qlo = g * 128 if mask_mode == "causal" else 0
                        for pair in range(2):
                            kt0 = 4 * g + 2 * pair
                            expS = expp.tile([128, 2, 2, 512], FP8, tag="expS")
                            for par in range(2):
                                kt = kt0 + par
                                t_in_g = kt - 4 * g
                                has_bias = mask_mode in ("causal", "allbias")
                                psum_s = ps_s.tile([128, 2, 512], F32, tag="pss")
                                # even/odd heads live on partition halves
                                # 0:64 / 64:128 of KT/QT -> distinct PE row
                                # groups -> the two score matmuls execute
                                # concurrently.
                                for i, h in enumerate(h2):
                                    p0 = 64 * (h % 2)
                                    nc.tensor.matmul(
                                        psum_s[:, i, qlo:512],
                                        KT[ds(p0, DK), gh, ds(kt * 128, 128)],
                                        QT[ds(p0, DK), gh, qlo:512],
                                        start=True, stop=not has_bias,
                                    )
                                if mask_mode == "causal":
                                    for i in range(2):
                                        nc.tensor.matmul(
                                            psum_s[:, i, ds(g * 128, 128)],
                                            sa_bias_sb[:, g, 0, t_in_g, :],
                                            ident,
                                            start=False, stop=True,
                                            skip_group_check=True,
                                        )
                                elif mask_mode == "allbias":
                                    for i in range(2):
                                        for sl in range(QSUB):
                                            nc.tensor.matmul(
                                                psum_s[:, i, ds(sl * 128, 128)],
                                                sa_bias_sb[:, sl, g, t_in_g, :],
                                                ident,
                                                start=False, stop=(sl == QSUB - 1),
                                                skip_group_check=True,
                                            )
                                if use_ca_kbias:
                                    nc.vector.tensor_scalar(
                                        out=psum_s[:, :, qlo:512],
                                        in0=psum_s[:, :, qlo:512],
                                        scalar1=ca_kb_sb[:, kt:kt + 1],
                                        scalar2=None,
                                        op0=mybir.AluOpType.add,
                                    )
                                nc.scalar.activation(
                                    out=expS[:, par, :, qlo:512],
                                    in_=psum_s[:, :, qlo:512],
                                    func=mybir.ActivationFunctionType.Exp,
                                )
                            for i, h in enumerate(h2):
                                nc.tensor.matmul(
                                    psum_os[i][:, qlo:512],
                                    V[:, kt0:kt0 + 2, h, 0:DK + 1],
                                    expS[:, :, i, qlo:512],
                                    start=(kt0 == 0), stop=(kt0 == n_kt - 2),
                                    perf_mode=DR,
                                )
                    # softmax denominator: reciprocal in place at partition
                    # 64, broadcast across 64 partitions with a K=1 matmul,
                    # normalize fused with the PSUM->SBUF copy (out fp8
                    # carries the x64 dynamic-range scale cancelled by V's).
                    for i, h in enumerate(h2):
                        rh = rhp.tile([DK + 1, 512], BF16, tag="rh")
                        with nc.allow_low_precision(reason="softmax denom bf16"):
                            nc.vector.reciprocal(
                                out=rh[DK:DK + 1, :], in_=psum_os[i][DK:DK + 1, :]
                            )
                        psum_r = ps.tile([DK, 512], F32, tag="psg")
                        nc.tensor.matmul(
                            psum_r, ones65[DK:DK + 1, :], rh[DK:DK + 1, :],
                            start=True, stop=True,
                        )
                        rbc = work.tile([DK, 512], BF16, tag="rbc")
                        nc.vector.tensor_copy(out=rbc, in_=psum_r)
                        nc.vector.tensor_tensor(
                            out=attnT[:, h, :], in0=psum_os[i][0:DK, :],
                            in1=rbc, op=mybir.AluOpType.mult,
                        )
                    if filler:
                        filler.pop(0)()
                # output projection (DoubleRow over head pairs) + residual
                # (host- or LN-scale matched) + LN
                for s in range(QSUB):
                    psum = ps.tile([128, 512], F32, tag="psg")
                    for hp in range(H // 2):
                        nc.tensor.matmul(
                            psum, attnT[:, 2 * hp:2 * hp + 2, ds(s * 128, 128)],
                            wo8_sb[:, 2 * hp:2 * hp + 2, :],
                            start=(hp == 0), stop=(hp == H // 2 - 1),
                            perf_mode=DR,
                        )
                    tmp = work.tile([128, D], F32, tag="epi")
                    nc.vector.tensor_tensor(
                        out=tmp, in0=psum, in1=resid_sb[:, s, :],
                        op=mybir.AluOpType.add,
                    )
                    if bo_bc is not None:
                        nc.vector.tensor_tensor(
                            out=tmp, in0=tmp, in1=bo_bc, op=mybir.AluOpType.add,
                        )
                    layer_norm(tmp, x_out[:, s, :], ln_idx)

            def transpose_x(x_f32, xT_dst):
                """[128, QSUB, D] f32 (XS-scaled) -> bf16 -> feature-
                transposed fp8 [128, 4, 512] (unscaled)."""
                xbf = xbfp.tile([128, QSUB, D], BF16, tag="xbf")
                nc.gpsimd.tensor_copy(out=xbf, in_=x_f32)
                for s in range(QSUB):
                    for g in range(4):
                        pt = ps.tile([128, 128], BF16, tag="psg")
                        nc.tensor.transpose(pt, xbf[:, s, ds(g * 128, 128)], ident)
                        nc.vector.tensor_scalar(
                            out=xT_dst[:, g, ds(s * 128, 128)], in0=pt,
                            scalar1=1.0 / XS, scalar2=None,
                            op0=mybir.AluOpType.mult,
                        )

            def load_w4(names):
                tiles = []
                for i, nm in enumerate(names):
                    t = wp.tile([128, 4, D], FP8, tag=f"w4_{nm}")
                    nc.sync.dma_start(
                        out=t, in_=inp[nm][:].rearrange("(g p) n -> p g n", p=128)
                    )
                    tiles.append(t)
                return tiles

            def load_wo8(nm):
                t = wp.tile([DK, H, D], FP8, tag=f"wo8_{nm}")
                nc.sync.dma_start(out=t, in_=inp[nm][:])
                return t

            # ================= tile declarations =================
            KT_sa = attn.tile([128, 4, S], BF16, tag="KT_sa")
            V_sa = attn.tile([128, 16, H, 72], FP8, tag="V_sa")
            QT_sa = attn.tile([128, 4, 512], BF16, tag="QT_sa")
            KT_ca = attn.tile([128, 4, S_ENC], BF16, tag="KT_ca")
            V_ca = attn.tile([128, 16, H, 72], FP8, tag="V_ca")
            QT_ca = attn.tile([128, 4, 512], BF16, tag="QT_ca")
            attnT = attn.tile([DK, H, 512], FP8, tag="attnT")

            # ================= self-attention =================
            decT_sb = xt.tile([128, 4, S], FP8, tag="decT")
            decT_r = inp["decT"][:].rearrange("(g p) s -> p g s", p=128)
            for rg in range(4):
                nc.sync.dma_start(
                    out=decT_sb[:, :, ds(rg * 512, 512)],
                    in_=decT_r[:, :, ds(rg * 512, 512)],
                )
            qrhs = xt.tile([128, 4, 512], FP8, tag="q_rhs")
            nc.sync.dma_start(
                out=qrhs, in_=inp["qT0"][:].rearrange("(g p) s -> p g s", p=128)
            )
            wq_sa, wk_sa, wv_sa = load_w4(["w_sa_q", "w_sa_k", "w_sa_v"])
            wo8_sa = load_wo8("w_sa_o8")
            nc.vector.memset(V_sa[:, :, :, DK:DK + 1], 1.0)
            for rg in range(4):
                kv_projection_rg(KT_sa, V_sa, decT_sb, wk_sa, wv_sa, rg)
            q_projection(QT_sa, qrhs, wq_sa, bq_sa_sb)

            # CA inputs + weights issued now; projection work is emitted as
            # filler inside the SA attention loop.
            encT_sb = xt.tile([128, 4, S_ENC], FP8, tag="encT")
            nc.sync.dma_start(
                out=encT_sb, in_=inp["encT"][:].rearrange("(g p) s -> p g s", p=128)
            )
            wq_ca, wk_ca, wv_ca = load_w4(["w_ca_q", "w_ca_k", "w_ca_v"])
            wo8_ca = load_wo8("w_ca_o8")
            nc.vector.memset(V_ca[:, :, :, DK:DK + 1], 1.0)
            filler = [
                (lambda rg=rg: kv_projection_rg(
                    KT_ca, V_ca, encT_sb, wk_ca, wv_ca, rg))
                for rg in range(4)
            ]

            sa_mode = "allbias" if sa_all_bias else "causal"
            attention(KT_sa, V_sa, QT_sa, attnT, sa_mode, False, wo8_sa,
                      None, resid0_sb, x1, 1, filler)
            for f in filler:
                f()

            # ================= cross-attention =================
            x1T = xt.tile([128, 4, 512], FP8, tag="x1T")
            transpose_x(x1, x1T)
            q_projection(QT_ca, x1T, wq_ca, bq_ca_sb)

            w1_sb = wp.tile([128, 4, DFF], FP8, tag="w4_ff1")
            nc.sync.dma_start(
                out=w1_sb, in_=inp["w_ff1"][:].rearrange("(g p) n -> p g n", p=128)
            )
            w2_sb = wp.tile([128, 16, D], FP8, tag="w4_ff2")
            nc.sync.dma_start(
                out=w2_sb, in_=inp["w_ff2"][:].rearrange("(c p) n -> p c n", p=128)
            )

            attention(KT_ca, V_ca, QT_ca, attnT, "none", ca_kbias, wo8_ca,
                      bo_ca_bc, x1, x2, 2, [])

            # ================= FFN =================
            x2T = xt.tile([128, 4, 512], FP8, tag="x2T")
            transpose_x(x2, x2T)
            hT = attn.tile([128, 16, 512], FP8, tag="hT")
            for hc in range(16):
                psum = ps.tile([128, 512], F32, tag="psg")
                for gp in range(2):
                    nc.tensor.matmul(
                        psum, w1_sb[:, 2 * gp:2 * gp + 2, ds(hc * 128, 128)],
                        x2T[:, 2 * gp:2 * gp + 2, :],
                        start=(gp == 0), stop=(gp == 1), perf_mode=DR,
                    )
                # hT = max(64*(w1^T x2) + 64*b1, 0) = 64*relu(z)
                nc.vector.tensor_scalar(
                    out=hT[:, hc, :], in0=psum,
                    scalar1=bff1_sb[:, hc:hc + 1], scalar2=0.0,
                    op0=mybir.AluOpType.add, op1=mybir.AluOpType.max,
                )
            for s in range(QSUB):
                psum = ps.tile([128, 512], F32, tag="psg")
                for cp in range(8):
                    nc.tensor.matmul(
                        psum, hT[:, 2 * cp:2 * cp + 2, ds(s * 128, 128)],
                        w2_sb[:, 2 * cp:2 * cp + 2, :],
                        start=(cp == 0), stop=(cp == 7), perf_mode=DR,
                    )
                tmp = work.tile([128, D], F32, tag="epi")
                nc.vector.tensor_tensor(
                    out=tmp, in0=psum, in1=x2[:, s, :], op=mybir.AluOpType.add
                )
                nc.vector.tensor_tensor(
                    out=tmp, in0=tmp, in1=bff2_bc, op=mybir.AluOpType.add
                )
                x3 = work.tile([128, D], F32, tag="x3")
                layer_norm(tmp, x3[:], 3)
                nc.sync.dma_start(out=out_y[ds(s * 128, 128), :], in_=x3)

    return nc


# ---------------------------------------------------------------------------
# host side
# ---------------------------------------------------------------------------
def _fp8(a):
    return np.asarray(a, dtype=ml_dtypes.float8_e4m3)


def _bf16(a):
    return np.asarray(a, dtype=ml_dtypes.bfloat16)


def _prep_core_inputs(core, inputs, ln_identity, sa_all_bias, ca_kbias):
    b, j = core // 4, core % 4
    qis = [j, 4 + j, 8 + j, 12 + j]
    dec = np.asarray(inputs["dec"], np.float32)
    enc = np.asarray(inputs["enc"], np.float32)
    tgt = np.asarray(inputs["tgt_mask"])  # [1,1,S,S] (broadcasts over batch)
    src = np.asarray(inputs["src_mask"])  # [B,1,1,S_ENC]

    m = {}
    m["identity"] = _bf16(np.eye(128, dtype=np.float32))
    m["decT"] = _fp8(dec[b].T.copy())
    rows = np.concatenate(
        [dec[b, qi * 128:(qi + 1) * 128, :] for qi in qis], axis=0
    )
    m["qT0"] = _fp8(rows.T.copy())
    bo_sa_full = (
        np.asarray(inputs["sa_bv"], np.float32) @ np.asarray(inputs["sa_wo"], np.float32)
        + np.asarray(inputs["sa_bo"], np.float32)
    )
    m["resid0"] = np.ascontiguousarray(XS * (rows + bo_sa_full), np.float32)
    m["encT"] = _fp8(enc[b].T.copy())

    for nm, key in [("w_sa_q", "sa_wq"), ("w_sa_k", "sa_wk"), ("w_sa_v", "sa_wv"),
                    ("w_ca_q", "ca_wq"), ("w_ca_k", "ca_wk"), ("w_ca_v", "ca_wv"),
                    ("w_ff1", "ffn_w1"), ("w_ff2", "ffn_w2")]:
        m[nm] = _fp8(WS * np.asarray(inputs[key], np.float32))
    for nm, key in [("w_sa_o8", "sa_wo"), ("w_ca_o8", "ca_wo")]:
        w = WS * np.asarray(inputs[key], np.float32)  # [512, 512]
        m[nm] = _fp8(w.reshape(H, DK, D).transpose(1, 0, 2).copy())
    m["bq_sa"] = WS * np.asarray(inputs["sa_bq"], np.float32)
    m["bq_ca"] = WS * np.asarray(inputs["ca_bq"], np.float32)
    m["bo_ca"] = XS * (
        np.asarray(inputs["ca_bv"], np.float32) @ np.asarray(inputs["ca_wo"], np.float32)
        + np.asarray(inputs["ca_bo"], np.float32)
    ).astype(np.float32)
    m["bff1"] = WS * np.asarray(inputs["ffn_b1"], np.float32)
    m["bff2"] = XS * np.asarray(inputs["ffn_b2"], np.float32)

    # SA additive bias tiles in [q, k] orientation (bias matmul lhsT).
    ngb = 4 if sa_all_bias else 1
    sa_bias = np.zeros((QSUB, ngb, 4, 128, 128), np.float32)
    tmask = np.asarray(tgt[0, 0])  # [S, S]; nonzero = visible
    for s, qi in enumerate(qis):
        qrows = slice(qi * 128, (qi + 1) * 128)
        for g in range(ngb):
            gg = s if not sa_all_bias else g
            for t in range(4):
                kt = 4 * gg + t
                blk = tmask[qrows, kt * 128:(kt + 1) * 128]
                sa_bias[s, g, t][blk == 0] = MASK_NEG
    m["sa_bias"] = _bf16(sa_bias)

    if ca_kbias:
        kb = np.zeros((KT_CA, 128), np.float32)
        smask = np.asarray(src[b, 0, 0]).reshape(KT_CA, 128)
        kb[smask == 0] = MASK_NEG
        m["ca_kb"] = kb

    if not ln_identity:
        for i in (1, 2, 3):
            m[f"ln{i}_g"] = np.asarray(inputs[f"ln{i}_g"], np.float32)
            scale = XS if i in (1, 2) else 1.0
            m[f"ln{i}_b"] = scale * np.asarray(inputs[f"ln{i}_b"], np.float32)
    return m


_prog_cache = {}


def kernel(**inputs):
    tgt = np.asarray(inputs["tgt_mask"])
    src = np.asarray(inputs["src_mask"])
    causal = bool(
        np.array_equal(tgt[0, 0], np.tril(np.ones((S, S), tgt.dtype)))
    )
    sa_all_bias = not causal
    ca_kbias = not bool((src != 0).all())
    ln_identity = all(
        np.allclose(inputs[f"ln{i}_g"], 1.0)
        and np.allclose(inputs[f"ln{i}_b"], 0.0)
        for i in (1, 2, 3)
    )

    key = (ln_identity, sa_all_bias, ca_kbias)
    if key not in _prog_cache:
        _prog_cache[key] = build_program(*key)
    nc = _prog_cache[key]

    in_maps = [
        _prep_core_inputs(c, inputs, ln_identity, sa_all_bias, ca_kbias)
        for c in range(NCORES)
    ]
    res = run_bass_kernel_spmd(nc, in_maps, core_ids=list(range(NCORES)))

    out = np.zeros((B, S, D), np.float32)
    for c in range(NCORES):
        b, j = c // 4, c % 4
        y = res.results[c]["y"]
        for s, qi in enumerate([j, 4 + j, 8 + j, 12 + j]):
            out[b, qi * 128:(qi + 1) * 128, :] = y[s * 128:(s + 1) * 128, :]
    return out


# revision 15
# speedup vs baseline: 1.3106x; 1.3106x over previous
"""Trainium2 Bass kernel for a transformer decoder layer (self-attn +
cross-attn + FFN) on 8 NeuronCores, zero collectives.

Sharding: data-parallel. Core c (0..7) owns batch b = c//4 and four
query subtiles qi in {j, 4+j, 8+j, 12+j} (j = c%4, 128 rows each) of
that batch — the stride-4 interleave balances the causal-attention
work across cores. Every core recomputes the full K/V projections for
its batch (2048 rows), so no inter-core communication is needed. The
compiled program is identical on all cores (SPMD); causal masking and
the per-core query positions live entirely in the data.

Performance layout (v3):
- Mixed precision: fp8e4 + DoubleRow matmuls (two k-tiles contracted
  per instruction) for the K/Q projections, the whole cross-attention
  block, and the FFN — paths whose quantization error is suppressed by
  softmax averaging or tiny relative to the residual. The SA V-path
  (V projection, AV, output projection) stays bf16: early causal rows
  attend to few keys, so nothing averages fp8's ~4% noise there.
- Weights are pre-scaled x64 on the host into fp8e4's normal range;
  activations carry a x4096 scale through the residual stream
  (LayerNorm is scale-invariant; the rescale folds into rstd).
- Scores are computed transposed (S^T [k, q]) in bf16; exp runs on the
  activation engine (fp8 out for CA, bf16 for SA); softmax skips the
  max trick (|scores| < ~2).
- Causal masking is multiplicative on exp(S) (exact zeros) and runs on
  the otherwise-idle GpSimd engine — no PE bias matmuls.
- The softmax denominator l[q] rides as a 65th "ones" column of V; it
  is copied out by the activation engine (Copy shares Exp's table),
  broadcast with a K=1 matmul, inverted with the fast DVE reciprocal
  approximation, and the normalize fuses into the PSUM->SBUF copy.
- SA K/V copies run on the activation engine (idle early); CA K/V
  projection is emitted interleaved with the SA head loop so the PE
  stays busy while attention is activation-bound.
"""

import contextlib

import numpy as np
import ml_dtypes

import concourse.bass as bass
import concourse.tile as tile
from concourse import mybir
from concourse.bass import ds
from concourse.bass_utils import run_bass_kernel_spmd

B, S, S_ENC, D, H, DK, DFF = 2, 2048, 2048, 512, 8, 64, 2048
EPS = 1e-5
NCORES = 8
QSUB = 4          # query subtiles per core (128 rows each)
KT_CA = 16        # cross-attention key tiles (128 keys each)
MASK_NEG = -30.0  # additive mask value (allbias fallback path)
WS = 64.0         # fp8 weight scale
XS = WS * WS      # activation carry scale (4096)

DEBUG = False

F32 = mybir.dt.float32
BF16 = mybir.dt.bfloat16
FP8 = mybir.dt.float8e4
DR = mybir.MatmulPerfMode.DoubleRow
AF = mybir.ActivationFunctionType


# ---------------------------------------------------------------------------
# walrus legalization: this neuronxcc build rejects instructions carrying
# more than one sync wait. Tile attaches several to the kernel-tail Drain
# (and occasionally elsewhere); hoist the extras onto same-engine NOPs.
# ---------------------------------------------------------------------------
def _split_multiwaits(nc):
    nopid = 0
    for fn in nc.m.functions:
        for blk in fn.blocks:
            insts = blk.instructions
            i = 0
            while i < len(insts):
                inst = insts[i]
                si = getattr(inst, "sync_info", None)
                if si is not None and len(si.on_wait) > 1:
                    waits = list(si.on_wait)
                    inst.sync_info = mybir.SyncInfo(
                        on_wait=[waits[-1]], on_update=list(si.on_update)
                    )
                    for w in waits[:-1]:
                        nop = mybir.InstNoOp(
                            name=f"I-waitsplit-{nopid}",
                            engine=inst.engine,
                            sync_info=mybir.SyncInfo(on_wait=[w], on_update=[]),
                            bass_nofuse=True,
                        )
                        nopid += 1
                        insts.insert(i, nop)
                        i += 1
                i += 1


class _TileContext(tile.TileContext):
    def __exit__(self, exc_type, exc, tb):
        ret = super().__exit__(exc_type, exc, tb)
        if exc_type is None:
            _split_multiwaits(self.nc)
        return ret


def _bcast_dram(dram_ap, parts=128):
    """AP reading a 1-D DRAM vector replicated across `parts` partitions."""
    return bass.AP(
        tensor=dram_ap.tensor,
        offset=dram_ap.offset,
        ap=[[0, parts]] + list(dram_ap.ap),
    )


# ---------------------------------------------------------------------------
# program builder (identical for every core; all core differences are data)
# ---------------------------------------------------------------------------
def build_program(ln_identity, sa_all_bias, ca_kbias):
    nc = bass.Bass()

    inp = {}

    def dram(name, shape, dt):
        inp[name] = nc.declare_dram_parameter(name, list(shape), dt, isOutput=False)
        return inp[name]

    dram("identity", (128, 128), BF16)
    dram("decT", (D, S), FP8)
    dram("qT0", (D, 512), FP8)
    dram("resid0", (512, D), BF16)     # XS*(dec rows + bv@wo + bo)
    dram("encT", (D, S_ENC), FP8)
    for wnm, shp, dt in [
        ("w_sa_q", (D, D), FP8), ("w_sa_k", (D, D), FP8),
        ("w_sa_v", (D, D), BF16),
        ("w_ca_q", (D, D), FP8), ("w_ca_k", (D, D), FP8),
        ("w_ca_v", (D, D), FP8),
        ("w_ff1", (D, DFF), FP8), ("w_ff2", (DFF, D), FP8),
    ]:
        dram(wnm, shp, dt)
    dram("w_sa_o8", (DK, H, D), BF16)  # wo reshaped to [64, head, 512]
    dram("w_ca_o8", (DK, H, D), FP8)   # 64*wo
    dram("bq_sa", (D,), F32)      # 64*bq
    dram("bq_ca", (D,), F32)
    dram("bo_ca", (D,), BF16)     # XS*(bv@wo + bo)
    dram("bff1", (DFF,), F32)     # 64*b1
    dram("bff2", (D,), BF16)      # XS*b2
    if sa_all_bias:
        # [slot, group, kt-in-group, q 128, k 128] bf16, [q, k] orientation
        dram("sa_bias", (QSUB, 4, 4, 128, 128), BF16)
    else:
        # multiplicative {0,1} mask for the diagonal groups, head-duplicated
        dram("sa_mask", (4, 4, 2, 128, 128), BF16)  # [g, t, hdup, k, q]
    if ca_kbias:
        dram("ca_kb", (KT_CA, 128), F32)
    if not ln_identity:
        for i in (1, 2, 3):
            dram(f"ln{i}_g", (D,), F32)
            dram(f"ln{i}_b", (D,), F32)   # ln1/ln2 host-scaled by XS

    out_y = nc.declare_dram_parameter("y", [512, D], F32, isOutput=True)
    if DEBUG:
        dbg = {
            "d_kt": nc.declare_dram_parameter("d_kt", [128, 4, 512], BF16, isOutput=True),
            "d_v": nc.declare_dram_parameter("d_v", [128, H, DK + 1], BF16, isOutput=True),
            "d_at": nc.declare_dram_parameter("d_at", [DK, H, 512], BF16, isOutput=True),
            "d_x1": nc.declare_dram_parameter("d_x1", [128, QSUB, D], F32, isOutput=True),
            "d_qt": nc.declare_dram_parameter("d_qt", [128, 4, 512], BF16, isOutput=True),
        }

    with _TileContext(nc) as tc:
        with contextlib.ExitStack() as ctx:
            const = ctx.enter_context(tc.tile_pool(name="const", bufs=1))
            xt = ctx.enter_context(tc.tile_pool(name="xt", bufs=1))
            wp = ctx.enter_context(tc.tile_pool(name="wp", bufs=1))
            res = ctx.enter_context(tc.tile_pool(name="res", bufs=1))
            work = ctx.enter_context(tc.tile_pool(name="work", bufs=2))
            xbfp = ctx.enter_context(tc.tile_pool(name="xbfp", bufs=1))
            attn = ctx.enter_context(tc.tile_pool(name="attn", bufs=1))
            expp = ctx.enter_context(tc.tile_pool(name="expp", bufs=3))
            rhp = ctx.enter_context(tc.tile_pool(name="rhp", bufs=2))
            ps = ctx.enter_context(tc.tile_pool(name="ps", bufs=2, space="PSUM"))
            ps_s = ctx.enter_context(tc.tile_pool(name="ps_s", bufs=2, space="PSUM"))
            ps_o = ctx.enter_context(tc.tile_pool(name="ps_o", bufs=2, space="PSUM"))

            # ---- constants ----
            ident = const.tile([128, 128], BF16)
            nc.sync.dma_start(out=ident, in_=inp["identity"][:])
            eps_t = const.tile([128, 1], F32)
            nc.vector.memset(eps_t, EPS)
            eps3_t = const.tile([128, 1], F32)
            nc.vector.memset(eps3_t, EPS * XS * XS)
            ones65 = const.tile([DK + 1, DK], BF16)
            nc.vector.memset(ones65, 1.0)
            ln64_t = const.tile([128, 1], F32)
            nc.vector.memset(ln64_t, float(np.log(WS)))

            def load_bc(name, dt=F32):
                t = const.tile([128, D], dt, tag=f"bc_{name}")
                nc.sync.dma_start(out=t, in_=_bcast_dram(inp[name][:]))
                return t

            bo_ca_bc = load_bc("bo_ca", BF16)
            bff2_bc = load_bc("bff2", BF16)
            ln_bc = {}
            if not ln_identity:
                for i in (1, 2, 3):
                    ln_bc[i] = (load_bc(f"ln{i}_g"), load_bc(f"ln{i}_b"))

            bq_sa_sb = const.tile([128, 4], F32)
            nc.sync.dma_start(
                out=bq_sa_sb, in_=inp["bq_sa"][:].rearrange("(g p) -> p g", p=128)
            )
            bq_ca_sb = const.tile([128, 4], F32)
            nc.sync.dma_start(
                out=bq_ca_sb, in_=inp["bq_ca"][:].rearrange("(g p) -> p g", p=128)
            )
            bff1_sb = const.tile([128, 16], F32)
            nc.sync.dma_start(
                out=bff1_sb, in_=inp["bff1"][:].rearrange("(c p) -> p c", p=128)
            )

            if sa_all_bias:
                sa_bias_sb = const.tile([128, QSUB, 4, 4, 128], BF16)
                nc.sync.dma_start(
                    out=sa_bias_sb,
                    in_=inp["sa_bias"][:].rearrange("s g t p k -> p s g t k"),
                )
            else:
                sa_mask_sb = const.tile([128, 4, 4, 2, 128], BF16)
                nc.sync.dma_start(
                    out=sa_mask_sb,
                    in_=inp["sa_mask"][:].rearrange("g t e p q -> p g t e q"),
                )
            if ca_kbias:
                ca_kb_sb = const.tile([128, KT_CA], F32)
                nc.sync.dma_start(
                    out=ca_kb_sb, in_=inp["ca_kb"][:].rearrange("t p -> p t")
                )

            x1 = res.tile([128, QSUB, D], F32, tag="x1")
            x2 = res.tile([128, QSUB, D], F32, tag="x2")
            resid0_sb = res.tile([128, QSUB, D], BF16, tag="r0")
            nc.sync.dma_start(
                out=resid0_sb,
                in_=inp["resid0"][:].rearrange("(s p) d -> p s d", p=128),
            )

            # =============================================================
            def k_projection_rg(KT_t, srcT_sb, wk_sb, rg, copy_eng):
                """One 512-key row group of the K projection (fp8 DoubleRow)."""
                for go in range(4):
                    psum = ps.tile([128, 512], F32, tag="psg")
                    for gp in range(2):
                        nc.tensor.matmul(
                            psum, wk_sb[:, 2 * gp:2 * gp + 2, ds(go * 128, 128)],
                            srcT_sb[:, 2 * gp:2 * gp + 2, ds(rg * 512, 512)],
                            start=(gp == 0), stop=(gp == 1), perf_mode=DR,
                        )
                    if copy_eng == "act":
                        nc.scalar.activation(
                            out=KT_t[:, go, ds(rg * 512, 512)], in_=psum,
                            func=AF.Copy,
                        )
                    else:
                        nc.vector.tensor_copy(
                            out=KT_t[:, go, ds(rg * 512, 512)], in_=psum
                        )

            def v_projection_rg_bf(V_t, srcTb_sb, wv_sb, rg, copy_eng):
                """One row group of the SA V projection (bf16)."""
                for k2 in range(4):
                    kc = rg * 4 + k2
                    psum = ps.tile([128, 512], F32, tag="psg")
                    for gi in range(4):
                        nc.tensor.matmul(
                            psum, srcTb_sb[:, gi, ds(kc * 128, 128)],
                            wv_sb[:, gi, :],
                            start=(gi == 0), stop=(gi == 3),
                        )
                    if copy_eng == "act":
                        nc.scalar.activation(
                            out=V_t[:, kc, :, 0:DK],
                            in_=psum.rearrange("p (h d) -> p h d", h=H),
                            func=AF.Copy,
                        )
                    else:
                        nc.vector.tensor_copy(
                            out=V_t[:, kc, :, 0:DK],
                            in_=psum.rearrange("p (h d) -> p h d", h=H),
                        )

            def v_projection_rg_fp8(V_t, srcT_sb, wv_sb, rg):
                """One row group of the CA V projection (fp8 DoubleRow)."""
                for k2 in range(4):
                    kc = rg * 4 + k2
                    psum = ps.tile([128, 512], F32, tag="psg")
                    for gp in range(2):
                        nc.tensor.matmul(
                            psum, srcT_sb[:, 2 * gp:2 * gp + 2, ds(kc * 128, 128)],
                            wv_sb[:, 2 * gp:2 * gp + 2, :],
                            start=(gp == 0), stop=(gp == 1), perf_mode=DR,
                        )
                    nc.vector.tensor_copy(
                        out=V_t[:, kc, :, 0:DK],
                        in_=psum.rearrange("p (h d) -> p h d", h=H),
                    )

            def q_projection(QT_t, q_rhs_sb, wq_sb, bq_sb):
                for go in range(4):
                    psum = ps.tile([128, 512], F32, tag="psg")
                    for gp in range(2):
                        nc.tensor.matmul(
                            psum, wq_sb[:, 2 * gp:2 * gp + 2, ds(go * 128, 128)],
                            q_rhs_sb[:, 2 * gp:2 * gp + 2, :],
                            start=(gp == 0), stop=(gp == 1), perf_mode=DR,
                        )
                    # QT = (64*(wq^T x) + 64*bq) / (64*64*sqrt(dk)) = q/512
                    nc.vector.tensor_scalar(
                        out=QT_t[:, go, :], in0=psum,
                        scalar1=bq_sb[:, go:go + 1],
                        scalar2=1.0 / (WS * WS * np.sqrt(DK)),
                        op0=mybir.AluOpType.add, op1=mybir.AluOpType.mult,
                    )

            def layer_norm(src_sb, dst_ap, ln_idx):
                """src = XS*u; LN1/2 write XS*ln(u), LN3 writes ln(u)."""
                stats = work.tile([128, 6], F32, tag="lnstats")
                nc.vector.bn_stats(out=stats, in_=src_sb)
                mv = work.tile([128, 2], F32, tag="lnmv")
                nc.vector.bn_aggr(out=mv, in_=stats)
                lnv = work.tile([128, 1], F32, tag="lnlnv")
                last = ln_idx == 3
                nc.scalar.activation(
                    out=lnv, in_=mv[:, 1:2],
                    func=AF.Ln,
                    bias=eps3_t if last else eps_t,
                    scale=1.0 if last else 1.0 / (XS * XS),
                )
                rstd = work.tile([128, 1], F32, tag="lnrstd")
                nc.scalar.activation(
                    out=rstd, in_=lnv, func=AF.Exp, scale=-0.5,
                )
                nc.vector.tensor_scalar(
                    out=dst_ap, in0=src_sb,
                    scalar1=mv[:, 0:1], scalar2=rstd,
                    op0=mybir.AluOpType.subtract, op1=mybir.AluOpType.mult,
                )
                if not ln_identity:
                    g_bc, b_bc = ln_bc[ln_idx]
                    nc.vector.tensor_tensor(
                        out=dst_ap, in0=dst_ap, in1=g_bc, op=mybir.AluOpType.mult
                    )
                    nc.vector.tensor_tensor(
                        out=dst_ap, in0=dst_ap, in1=b_bc, op=mybir.AluOpType.add
                    )

            def denom_normalize(psum_os, h2, attnT, scale64):
                """attnT <- (WS*AV)/l. 1/l = exp(-ln(l)) on the activation
                engine (ln/exp share a table set - no switches), broadcast
                across 64 partitions with a K=1 matmul, normalize fused with
                the PSUM->SBUF copy. scale64: multiply by WS via exp's bias
                (SA's bf16 V carries no weight scale; CA's fp8 V does)."""
                for i, h in enumerate(h2):
                    lrow = rhp.tile([DK + 1, 512], F32, tag="lrow")
                    nc.scalar.activation(
                        out=lrow[DK:DK + 1, :], in_=psum_os[i][DK:DK + 1, :],
                        func=AF.Ln,
                    )
                    rrow = rhp.tile([DK + 1, 512], BF16, tag="rrow")
                    if scale64:
                        nc.scalar.activation(
                            out=rrow[DK:DK + 1, :], in_=lrow[DK:DK + 1, :],
                            func=AF.Exp, scale=-1.0, bias=ln64_t[DK:DK + 1, :],
                        )
                    else:
                        nc.scalar.activation(
                            out=rrow[DK:DK + 1, :], in_=lrow[DK:DK + 1, :],
                            func=AF.Exp, scale=-1.0,
                        )
                    psum_r = ps.tile([DK, 512], F32, tag="psg")
                    nc.tensor.matmul(
                        psum_r, ones65[DK:DK + 1, :], rrow[DK:DK + 1, :],
                        start=True, stop=True,
                    )
                    rbc = work.tile([DK, 512], BF16, tag="rbc")
                    nc.vector.tensor_copy(out=rbc, in_=psum_r)
                    nc.vector.tensor_tensor(
                        out=attnT[:, h, :], in0=psum_os[i][0:DK, :],
                        in1=rbc, op=mybir.AluOpType.mult,
                    )

            def out_projection(attnT, wo8_sb, fp8, bo_bc, resid_sb, x_out,
                               ln_idx):
                for s in range(QSUB):
                    psum = ps.tile([128, 512], F32, tag="psg")
                    if fp8:
                        for hp in range(H // 2):
                            nc.tensor.matmul(
                                psum,
                                attnT[:, 2 * hp:2 * hp + 2, ds(s * 128, 128)],
                                wo8_sb[:, 2 * hp:2 * hp + 2, :],
                                start=(hp == 0), stop=(hp == H // 2 - 1),
                                perf_mode=DR,
                            )
                    else:
                        for h in range(H):
                            nc.tensor.matmul(
                                psum, attnT[:, h, ds(s * 128, 128)],
                                wo8_sb[:, h, :],
                                start=(h == 0), stop=(h == H - 1),
                            )
                    tmp = work.tile([128, D], F32, tag="epi")
                    nc.vector.tensor_tensor(
                        out=tmp, in0=psum, in1=resid_sb[:, s, :],
                        op=mybir.AluOpType.add,
                    )
                    if bo_bc is not None:
                        nc.vector.tensor_tensor(
                            out=tmp, in0=tmp, in1=bo_bc, op=mybir.AluOpType.add,
                        )
                    layer_norm(tmp, x_out[:, s, :], ln_idx)

            def attention_sa(KT, V, QT, attnT, filler):
                """bf16 V-path self-attention. Causal mask is multiplicative
                on exp(S) via GpSimd; allbias falls back to PE bias matmuls.
                filler: thunks emitted between head-pairs (independent PE
                work to fill the ACT-bound loop)."""
                causal = not sa_all_bias
                for hp in range(H // 2):
                    h2 = (2 * hp, 2 * hp + 1)
                    psum_os = [
                        ps_o.tile([DK + 1, 512], F32, tag="po", name=f"sa_po_{hp}_{i}")
                        for i in range(2)
                    ]
                    for g in range(4):
                        qlo = g * 128 if causal else 0
                        for kt in range(4 * g, 4 * g + 4):
                            t = kt - 4 * g
                            psum_s = ps_s.tile([128, 2, 512], F32, tag="pss")
                            for i, h in enumerate(h2):
                                p0 = 64 * (h % 2)
                                nc.tensor.matmul(
                                    psum_s[:, i, qlo:512],
                                    KT[ds(p0, DK), hp, ds(kt * 128, 128)],
                                    QT[ds(p0, DK), hp, qlo:512],
                                    start=True, stop=causal,
                                )
                            if not causal:
                                for i in range(2):
                                    for sl in range(QSUB):
                                        nc.tensor.matmul(
                                            psum_s[:, i, ds(sl * 128, 128)],
                                            sa_bias_sb[:, sl, g, t, :],
                                            ident,
                                            start=False, stop=(sl == QSUB - 1),
                                            skip_group_check=True,
                                        )
                            expS = expp.tile([128, 2, 512], BF16, tag="expS_sa")
                            nc.scalar.activation(
                                out=expS[:, :, qlo:512],
                                in_=psum_s[:, :, qlo:512],
                                func=AF.Exp,
                            )
                            if causal:
                                # zero the masked part of the diagonal block
                                nc.gpsimd.tensor_tensor(
                                    out=expS[:, :, ds(g * 128, 128)],
                                    in0=expS[:, :, ds(g * 128, 128)],
                                    in1=sa_mask_sb[:, g, t, :, :],
                                    op=mybir.AluOpType.mult,
                                )
                            for i, h in enumerate(h2):
                                nc.tensor.matmul(
                                    psum_os[i][:, qlo:512],
                                    V[:, kt, h, :],
                                    expS[:, i, qlo:512],
                                    start=(kt == 0), stop=(kt == 15),
                                )
                    denom_normalize(psum_os, h2, attnT, True)
                    if filler:
                        filler.pop(0)()

            def attention_ca(KT, V, QT, attnT):
                """fp8 DoubleRow cross-attention (no mask / per-key bias)."""
                for hp in range(H // 2):
                    h2 = (2 * hp, 2 * hp + 1)
                    psum_os = [
                        ps_o.tile([DK + 1, 512], F32, tag="po", name=f"ca_po_{hp}_{i}")
                        for i in range(2)
                    ]
                    for pair in range(8):
                        kt0 = 2 * pair
                        expS = expp.tile([128, 2, 2, 512], FP8, tag="expS_ca")
                        for par in range(2):
                            kt = kt0 + par
                            psum_s = ps_s.tile([128, 2, 512], F32, tag="pss")
                            for i, h in enumerate(h2):
                                p0 = 64 * (h % 2)
                                nc.tensor.matmul(
                                    psum_s[:, i, :],
                                    KT[ds(p0, DK), hp, ds(kt * 128, 128)],
                                    QT[ds(p0, DK), hp, :],
                                    start=True, stop=not ca_kbias,
                                )
                            if ca_kbias:
                                nc.vector.tensor_scalar(
                                    out=psum_s, in0=psum_s,
                                    scalar1=ca_kb_sb[:, kt:kt + 1],
                                    scalar2=None,
                                    op0=mybir.AluOpType.add,
                                )
                            nc.scalar.activation(
                                out=expS[:, par, :, :], in_=psum_s,
                                func=AF.Exp,
                            )
                        for i, h in enumerate(h2):
                            nc.tensor.matmul(
                                psum_os[i],
                                V[:, kt0:kt0 + 2, h, 0:DK + 1],
                                expS[:, :, i, :],
                                start=(kt0 == 0), stop=(kt0 == 14),
                                perf_mode=DR,
                            )
                    denom_normalize(psum_os, h2, attnT, False)

            def transpose_x(x_f32, xT_dst):
                """[128, QSUB, D] f32 (XS-scaled) -> bf16 -> feature-
                transposed fp8 [128, 4, 512] (unscaled)."""
                xbf = xbfp.tile([128, QSUB, D], BF16, tag="xbf")
                nc.vector.tensor_copy(out=xbf, in_=x_f32)
                for s in range(QSUB):
                    for g in range(4):
                        pt = ps.tile([128, 128], BF16, tag="psg")
                        nc.tensor.transpose(pt, xbf[:, s, ds(g * 128, 128)], ident)
                        nc.vector.tensor_scalar(
                            out=xT_dst[:, g, ds(s * 128, 128)], in0=pt,
                            scalar1=1.0 / XS, scalar2=None,
                            op0=mybir.AluOpType.mult,
                        )

            def load_w4(nm, dt):
                t = wp.tile([128, 4, D], dt, tag=f"w4_{nm}")
                nc.sync.dma_start(
                    out=t, in_=inp[nm][:].rearrange("(g p) n -> p g n", p=128)
                )
                return t

            def load_wo8(nm, dt):
                t = wp.tile([DK, H, D], dt, tag=f"wo8_{nm}")
                nc.sync.dma_start(out=t, in_=inp[nm][:])
                return t

            # ================= tile declarations =================
            KT_sa = attn.tile([128, 4, S], BF16, tag="KT_sa")
            V_sa = attn.tile([128, 16, H, DK + 1], BF16, tag="V_sa")
            QT_sa = attn.tile([128, 4, 512], BF16, tag="QT_sa")
            KT_ca = attn.tile([128, 4, S_ENC], BF16, tag="KT_ca")
            V_ca = attn.tile([128, 16, H, 72], FP8, tag="V_ca")
            QT_ca = attn.tile([128, 4, 512], BF16, tag="QT_ca")
            attnT = attn.tile([DK, H, 512], BF16, tag="attnT")

            # ================= self-attention =================
            decT_sb = xt.tile([128, 4, S], FP8, tag="decT")
            decT_r = inp["decT"][:].rearrange("(g p) s -> p g s", p=128)
            for rg in range(4):
                nc.sync.dma_start(
                    out=decT_sb[:, :, ds(rg * 512, 512)],
                    in_=decT_r[:, :, ds(rg * 512, 512)],
                )
            qrhs = xt.tile([128, 4, 512], FP8, tag="q_rhs")
            nc.sync.dma_start(
                out=qrhs, in_=inp["qT0"][:].rearrange("(g p) s -> p g s", p=128)
            )
            wq_sa = load_w4("w_sa_q", FP8)
            wk_sa = load_w4("w_sa_k", FP8)
            wv_sa = load_w4("w_sa_v", BF16)
            wo8_sa = load_wo8("w_sa_o8", BF16)
            nc.vector.memset(V_sa[:, :, :, DK:DK + 1], 1.0)
            for rg in range(4):
                k_projection_rg(KT_sa, decT_sb, wk_sa, rg, "act")
                v_projection_rg_bf(V_sa, decT_sb, wv_sa, rg, "act")
            q_projection(QT_sa, qrhs, wq_sa, bq_sa_sb)

            # CA inputs + weights issued now; projection work is emitted as
            # filler inside the SA attention loop.
            encT_sb = xt.tile([128, 4, S_ENC], FP8, tag="encT")
            nc.sync.dma_start(
                out=encT_sb, in_=inp["encT"][:].rearrange("(g p) s -> p g s", p=128)
            )
            wq_ca = load_w4("w_ca_q", FP8)
            wk_ca = load_w4("w_ca_k", FP8)
            wv_ca = load_w4("w_ca_v", FP8)
            wo8_ca = load_wo8("w_ca_o8", FP8)
            nc.vector.memset(V_ca[:, :, :, DK:DK + 1], 1.0)
            filler = [
                (lambda rg=rg: (
                    k_projection_rg(KT_ca, encT_sb, wk_ca, rg, "vec"),
                    v_projection_rg_fp8(V_ca, encT_sb, wv_ca, rg),
                ))
                for rg in range(4)
            ]

            attention_sa(KT_sa, V_sa, QT_sa, attnT, filler)
            for f in filler:
                f()
            if DEBUG:
                nc.sync.dma_start(out=dbg["d_kt"][:], in_=KT_sa[:, :, 0:512])
                nc.sync.dma_start(out=dbg["d_v"][:], in_=V_sa[:, 0, :, :])
                nc.sync.dma_start(out=dbg["d_at"][:], in_=attnT[:])
                nc.sync.dma_start(out=dbg["d_qt"][:], in_=QT_sa[:])
            out_projection(attnT, wo8_sa, False, None, resid0_sb, x1, 1)
            if DEBUG:
                nc.sync.dma_start(out=dbg["d_x1"][:], in_=x1)

            # ================= cross-attention =================
            x1T = xt.tile([128, 4, 512], FP8, tag="xT")
            transpose_x(x1, x1T)
            q_projection(QT_ca, x1T, wq_ca, bq_ca_sb)

            w1_sb = wp.tile([128, 4, DFF], FP8, tag="w4_ff1")
            nc.sync.dma_start(
                out=w1_sb, in_=inp["w_ff1"][:].rearrange("(g p) n -> p g n", p=128)
            )
            w2_sb = wp.tile([128, 16, D], FP8, tag="w4_ff2")
            nc.sync.dma_start(
                out=w2_sb, in_=inp["w_ff2"][:].rearrange("(c p) n -> p c n", p=128)
            )

            attention_ca(KT_ca, V_ca, QT_ca, attnT)
            out_projection(attnT, wo8_ca, False, bo_ca_bc, x1, x2, 2)

            # ================= FFN =================
            x2T = xt.tile([128, 4, 512], FP8, tag="xT")
            transpose_x(x2, x2T)
            hT = attn.tile([128, 16, 512], FP8, tag="hT")
            for hc in range(16):
                psum = ps.tile([128, 512], F32, tag="psg")
                for gp in range(2):
                    nc.tensor.matmul(
                        psum, w1_sb[:, 2 * gp:2 * gp + 2, ds(hc * 128, 128)],
                        x2T[:, 2 * gp:2 * gp + 2, :],
                        start=(gp == 0), stop=(gp == 1), perf_mode=DR,
                    )
                # hT = max(64*(w1^T x2) + 64*b1, 0) = 64*relu(z)
                nc.vector.tensor_scalar(
                    out=hT[:, hc, :], in0=psum,
                    scalar1=bff1_sb[:, hc:hc + 1], scalar2=0.0,
                    op0=mybir.AluOpType.add, op1=mybir.AluOpType.max,
                )
            for s in range(QSUB):
                psum = ps.tile([128, 512], F32, tag="psg")
                for cp in range(8):
                    nc.tensor.matmul(
                        psum, hT[:, 2 * cp:2 * cp + 2, ds(s * 128, 128)],
                        w2_sb[:, 2 * cp:2 * cp + 2, :],
                        start=(cp == 0), stop=(cp == 7), perf_mode=DR,
                    )
                tmp = work.tile([128, D], F32, tag="epi")
                nc.vector.tensor_tensor(
                    out=tmp, in0=psum, in1=x2[:, s, :], op=mybir.AluOpType.add
                )
                nc.vector.tensor_tensor(
                    out=tmp, in0=tmp, in1=bff2_bc, op=mybir.AluOpType.add
                )
                x3 = work.tile([128, D], F32, tag="x3")
                layer_norm(tmp, x3[:], 3)
                nc.sync.dma_start(out=out_y[ds(s * 128, 128), :], in_=x3)

    return nc


# ---------------------------------------------------------------------------
# host side
# ---------------------------------------------------------------------------
def _fp8(a):
    return np.asarray(a, dtype=ml_dtypes.float8_e4m3)


def _bf16(a):
    return np.asarray(a, dtype=ml_dtypes.bfloat16)


def _prep_core_inputs(core, inputs, ln_identity, sa_all_bias, ca_kbias):
    b, j = core // 4, core % 4
    qis = [j, 4 + j, 8 + j, 12 + j]
    dec = np.asarray(inputs["dec"], np.float32)
    enc = np.asarray(inputs["enc"], np.float32)
    tgt = np.asarray(inputs["tgt_mask"])  # [1,1,S,S] (broadcasts over batch)
    src = np.asarray(inputs["src_mask"])  # [B,1,1,S_ENC]

    m = {}
    m["identity"] = _bf16(np.eye(128, dtype=np.float32))
    m["decT"] = _fp8(dec[b].T.copy())
    rows = np.concatenate(
        [dec[b, qi * 128:(qi + 1) * 128, :] for qi in qis], axis=0
    )
    m["qT0"] = _fp8(rows.T.copy())
    bo_sa_full = (
        np.asarray(inputs["sa_bv"], np.float32) @ np.asarray(inputs["sa_wo"], np.float32)
        + np.asarray(inputs["sa_bo"], np.float32)
    )
    m["resid0"] = _bf16(XS * (rows + bo_sa_full))
    m["encT"] = _fp8(enc[b].T.copy())

    for nm, key, ws, cv in [
        ("w_sa_q", "sa_wq", WS, _fp8), ("w_sa_k", "sa_wk", WS, _fp8),
        ("w_sa_v", "sa_wv", 1.0, _bf16),
        ("w_ca_q", "ca_wq", WS, _fp8), ("w_ca_k", "ca_wk", WS, _fp8),
        ("w_ca_v", "ca_wv", WS, _fp8),
        ("w_ff1", "ffn_w1", WS, _fp8), ("w_ff2", "ffn_w2", WS, _fp8),
    ]:
        m[nm] = cv(ws * np.asarray(inputs[key], np.float32))
    wo = WS * np.asarray(inputs["sa_wo"], np.float32)
    m["w_sa_o8"] = _bf16(wo.reshape(H, DK, D).transpose(1, 0, 2).copy())
    wo = WS * np.asarray(inputs["ca_wo"], np.float32)
    m["w_ca_o8"] = _fp8(wo.reshape(H, DK, D).transpose(1, 0, 2).copy())
    m["bq_sa"] = WS * np.asarray(inputs["sa_bq"], np.float32)
    m["bq_ca"] = WS * np.asarray(inputs["ca_bq"], np.float32)
    m["bo_ca"] = _bf16(XS * (
        np.asarray(inputs["ca_bv"], np.float32) @ np.asarray(inputs["ca_wo"], np.float32)
        + np.asarray(inputs["ca_bo"], np.float32)
    ))
    m["bff1"] = WS * np.asarray(inputs["ffn_b1"], np.float32)
    m["bff2"] = _bf16(XS * np.asarray(inputs["ffn_b2"], np.float32))

    tmask = np.asarray(tgt[0, 0])  # [S, S]; nonzero = visible
    if sa_all_bias:
        sa_bias = np.zeros((QSUB, 4, 4, 128, 128), np.float32)
        for s, qi in enumerate(qis):
            qrows = slice(qi * 128, (qi + 1) * 128)
            for g in range(4):
                for t in range(4):
                    kt = 4 * g + t
                    blk = tmask[qrows, kt * 128:(kt + 1) * 128]
                    sa_bias[s, g, t][blk == 0] = MASK_NEG
        m["sa_bias"] = _bf16(sa_bias)
    else:
        # multiplicative mask for the diagonal group: [g, t, hdup, k, q]
        sa_mask = np.zeros((4, 4, 2, 128, 128), np.float32)
        for g in range(4):
            qi = qis[g]
            qrows = slice(qi * 128, (qi + 1) * 128)
            for t in range(4):
                kt = 4 * g + t
                blk = tmask[qrows, kt * 128:(kt + 1) * 128]  # [q, k]
                sa_mask[g, t, :, :, :] = (blk != 0).T[None, :, :]
        m["sa_mask"] = _bf16(sa_mask)

    if ca_kbias:
        kb = np.zeros((KT_CA, 128), np.float32)
        smask = np.asarray(src[b, 0, 0]).reshape(KT_CA, 128)
        kb[smask == 0] = MASK_NEG
        m["ca_kb"] = kb

    if not ln_identity:
        for i in (1, 2, 3):
            m[f"ln{i}_g"] = np.asarray(inputs[f"ln{i}_g"], np.float32)
            scale = XS if i in (1, 2) else 1.0
            m[f"ln{i}_b"] = scale * np.asarray(inputs[f"ln{i}_b"], np.float32)
    return m


_prog_cache = {}


def kernel(**inputs):
    tgt = np.asarray(inputs["tgt_mask"])
    src = np.asarray(inputs["src_mask"])
    causal = bool(
        np.array_equal(tgt[0, 0], np.tril(np.ones((S, S), tgt.dtype)))
    )
    sa_all_bias = not causal
    ca_kbias = not bool((src != 0).all())
    ln_identity = all(
        np.allclose(inputs[f"ln{i}_g"], 1.0)
        and np.allclose(inputs[f"ln{i}_b"], 0.0)
        for i in (1, 2, 3)
    )

    key = (ln_identity, sa_all_bias, ca_kbias)
    if key not in _prog_cache:
        _prog_cache[key] = build_program(*key)
    nc = _prog_cache[key]

    in_maps = [
        _prep_core_inputs(c, inputs, ln_identity, sa_all_bias, ca_kbias)
        for c in range(NCORES)
    ]
    res = run_bass_kernel_spmd(nc, in_maps, core_ids=list(range(NCORES)))

    out = np.zeros((B, S, D), np.float32)
    for c in range(NCORES):
        b, j = c // 4, c % 4
        y = res.results[c]["y"]
        for s, qi in enumerate([j, 4 + j, 8 + j, 12 + j]):
            out[b, qi * 128:(qi + 1) * 128, :] = y[s * 128:(s + 1) * 128, :]
    return out
